# revision 2
# baseline (speedup 1.0000x reference)
"""MoE dispatcher kernel for Trainium2 (8 NeuronCores, expert-parallel).

Contract: kernel(**inputs) takes FULL inputs and returns the FULL output.

Strategy (expert-parallel, matches the sharding hint):
  - host: softmax(gate_logits) -> top-2 -> combine weights per (token, expert)
  - host "all-to-all dispatch": for expert e, gather its routed tokens,
    pre-scale rows by the combine weight (w * (x @ W) == (w*x) @ W), pad to a
    common capacity C, transpose to [D, C] so the device streams tokens along
    the free dim.  One expert per core.
  - device (per core): Y^T[D,C] = W[e]^T-free GEMM via PE array, tiled
    [128 x 512] PSUM accumulation over K=D.
  - host "all-to-all combine": scatter-add each expert's Y rows back to the
    token axis (plain add; weights were folded into x).
"""

import os

import numpy as np

N_CORES = 8
P = 128
NSPLIT = 512  # max moving-operand / PSUM-bank free dim (fp32)

# matmul input dtype: "float32", "float32r", or "bfloat16"
MM_DT = os.environ.get("BASS_MOE_DT", "float32")
# device output dtype
OUT_DT = os.environ.get("BASS_MOE_OUT_DT", "float32")

_prog_cache: dict = {}


def _np_dt(name):
    if name == "bfloat16":
        import ml_dtypes

        return ml_dtypes.bfloat16
    return np.float32


def _build_program(D: int, C: int, mm_dt_name: str, out_dt_name: str):
    import concourse.bacc as bacc
    import concourse.mybir as mybir
    import concourse.tile as tile

    mm_dt = getattr(mybir.dt, mm_dt_name)
    out_dt = getattr(mybir.dt, out_dt_name)
    KT = D // P  # k tiles (contraction)
    MT = D // P  # m tiles (output features)
    # n tiles over the token capacity
    n_tiles = []
    n0 = 0
    while n0 < C:
        n_tiles.append((n0, min(NSPLIT, C - n0)))
        n0 += NSPLIT

    nc = bacc.Bacc(None, target_bir_lowering=False)
    xt = nc.declare_dram_parameter("xt", [KT, P, C], mm_dt, isOutput=False)
    w = nc.declare_dram_parameter("w", [KT, P, D], mm_dt, isOutput=False)
    yt = nc.declare_dram_parameter("yt", [MT, P, C], out_dt, isOutput=True)

    with tile.TileContext(nc) as tc:
        with (
            tc.tile_pool(name="wpool", bufs=1) as wpool,
            tc.tile_pool(name="xpool", bufs=1) as xpool,
            tc.tile_pool(name="psum", bufs=8, space="PSUM") as psum_pool,
            tc.tile_pool(name="opool", bufs=6) as opool,
        ):
            w_sb = wpool.tile([P, KT, D], mm_dt)
            x_sb = xpool.tile([P, KT, C], mm_dt)
            # One big DMA for all weights (HWDGE issue cost ~0.6us each, so
            # batch instead of per-k transfers).
            nc.sync.dma_start(w_sb[:], w[:, :, :].rearrange("k p d -> p k d"))
            # Activations: one DMA per n-tile so the first column block lands
            # early and compute overlaps the remaining loads.
            for n0, nsz in n_tiles:
                nc.sync.dma_start(
                    x_sb[:, :, n0 : n0 + nsz],
                    xt[:, :, n0 : n0 + nsz].rearrange("k p n -> p k n"),
                )
            ev = 0
            for mi in range(MT):
                ot = opool.tile([P, C], out_dt, tag="ot")
                for n0, nsz in n_tiles:
                    ps = psum_pool.tile([P, nsz], mybir.dt.float32, tag="ps")
                    for k in range(KT):
                        nc.tensor.matmul(
                            ps[:],
                            lhsT=w_sb[:, k, mi * P : (mi + 1) * P],
                            rhs=x_sb[:, k, n0 : n0 + nsz],
                            start=(k == 0),
                            stop=(k == KT - 1),
                        )
                    # Alternate eviction engine so neither ACT nor DVE is the
                    # bottleneck.
                    if ev % 2 == 0:
                        nc.vector.tensor_copy(ot[:, n0 : n0 + nsz], ps[:])
                    else:
                        nc.scalar.copy(ot[:, n0 : n0 + nsz], ps[:])
                    ev += 1
                nc.sync.dma_start(yt[mi, :, :], ot[:])
    nc.compile()
    return nc


def kernel(hidden: np.ndarray, gate_logits: np.ndarray, W: np.ndarray) -> np.ndarray:
    from concourse.bass_utils import run_bass_kernel_spmd

    B, S, D = hidden.shape
    T, E = gate_logits.shape
    assert E == N_CORES
    x = np.ascontiguousarray(hidden.reshape(T, D).astype(np.float32))

    # --- routing on host (fp32, matches reference softmax/top-2) ---
    g = gate_logits.astype(np.float32)
    m = g.max(axis=-1, keepdims=True)
    p = np.exp(g - m)
    p /= p.sum(axis=-1, keepdims=True)
    # top-2 expert ids per token (argpartition then order the pair)
    top2 = np.argpartition(-p, 1, axis=-1)[:, :2]

    routed = [np.nonzero((top2 == e).any(axis=1))[0] for e in range(E)]
    counts = np.array([len(r) for r in routed])
    C = max(256, int(-(-counts.max() // P)) * P)  # capacity, multiple of 128

    mm_np = _np_dt(MM_DT)
    out_np = _np_dt(OUT_DT)
    KT = D // P

    in_maps = []
    for e in range(E):
        idx = routed[e]
        scale = p[idx, e].astype(np.float32)
        xe = x[idx] * scale[:, None]  # [cnt, D]
        xt_full = np.zeros((D, C), dtype=mm_np)
        xt_full[:, : len(idx)] = xe.T.astype(mm_np)
        w_full = W[e].astype(mm_np)  # [D, D]
        in_maps.append(
            {
                "xt": np.ascontiguousarray(xt_full.reshape(KT, P, C)),
                "w": np.ascontiguousarray(w_full.reshape(KT, P, D)),
            }
        )

    key = (D, C, MM_DT, OUT_DT)
    if key not in _prog_cache:
        _prog_cache[key] = _build_program(D, C, MM_DT, OUT_DT)
    nc = _prog_cache[key]

    res = run_bass_kernel_spmd(nc, in_maps, core_ids=list(range(N_CORES)))

    # --- combine on host ---
    out = np.zeros((T, D), dtype=np.float32)
    for e in range(E):
        idx = routed[e]
        ye_t = res.results[e]["yt"].reshape(D, C)  # Y^T
        out[idx] += ye_t[:, : len(idx)].T.astype(np.float32)
    return out.reshape(B, S, D)


# revision 3
# speedup vs baseline: 1.1881x; 1.1881x over previous
"""MoE dispatcher kernel for Trainium2 (8 NeuronCores, expert-parallel).

Contract: kernel(**inputs) takes FULL inputs and returns the FULL output.

Strategy (expert-parallel, matches the sharding hint):
  - host: softmax(gate_logits) -> top-2 -> combine weights per (token, expert)
  - host "all-to-all dispatch": for expert e, gather its routed tokens,
    pre-scale rows by the combine weight (w * (x @ W) == (w*x) @ W), pad to a
    common capacity C, transpose to [D, C] so the device streams tokens along
    the free dim.  One expert per core.
  - device (per core): Y^T[D,C] = W[e]^T @ X^T via PE array, tiled
    [128 x 512] PSUM accumulation over K=D.
  - host "all-to-all combine": scatter-add each expert's Y rows back to the
    token axis (plain add; weights were folded into x).

DRAM layouts are host-permuted so every DMA is fully contiguous per
partition:
  w   [KT, 128, D]        w[k, p, :]    = W[e][k*128 + p, :]
  xt  [NT, 128, KT*nsz]   xt[j, p, k, :] = X^T[k*128 + p, n0_j : n0_j+nsz]
  yt  [MT, 128, C]        yt[m, p, :]   = Y^T[m*128 + p, :]
"""

import os

import numpy as np

N_CORES = 8
P = 128
NSPLIT = 512  # max moving-operand / PSUM-bank free dim (fp32)

# matmul input dtype: "float32", "float32r", or "bfloat16"
MM_DT = os.environ.get("BASS_MOE_DT", "bfloat16")
# device output dtype
OUT_DT = os.environ.get("BASS_MOE_OUT_DT", "float32")
WARMUP_MM = int(os.environ.get("BASS_MOE_WARMUP", "0"))

_prog_cache: dict = {}


def _np_dt(name):
    if name == "bfloat16":
        import ml_dtypes

        return ml_dtypes.bfloat16
    return np.float32


def _n_tiles(C):
    out = []
    n0 = 0
    while n0 < C:
        out.append((n0, min(NSPLIT, C - n0)))
        n0 += NSPLIT
    return out


def _build_program(D: int, C: int, mm_dt_name: str, out_dt_name: str):
    import concourse.bacc as bacc
    import concourse.mybir as mybir
    import concourse.tile as tile

    mm_dt = getattr(mybir.dt, mm_dt_name)
    out_dt = getattr(mybir.dt, out_dt_name)
    KT = D // P  # k tiles (contraction)
    MT = D // P  # m tiles (output features)
    n_tiles = _n_tiles(C)
    NT = len(n_tiles)

    nc = bacc.Bacc(None, target_bir_lowering=False)
    xt = nc.declare_dram_parameter(
        "xt", [NT, P, KT * NSPLIT], mm_dt, isOutput=False
    )
    w = nc.declare_dram_parameter("w", [KT, P, D], mm_dt, isOutput=False)
    yt = nc.declare_dram_parameter("yt", [MT, P, C], out_dt, isOutput=True)

    with tile.TileContext(nc) as tc:
        with (
            tc.tile_pool(name="wpool", bufs=KT) as wpool,
            tc.tile_pool(name="xpool", bufs=NT) as xpool,
            tc.tile_pool(name="psum", bufs=8, space="PSUM") as psum_pool,
            tc.tile_pool(name="opool", bufs=4) as opool,
            tc.tile_pool(name="warm", bufs=2) as warmpool,
        ):
            if WARMUP_MM:
                wt = warmpool.tile([P, 64], mm_dt, tag="warm_w")
                nc.vector.memset(wt[:], 0.0)
                for i in range(WARMUP_MM):
                    wp = psum_pool.tile([P, 64], mybir.dt.float32, tag="warm_ps")
                    nc.tensor.matmul(wp[:], lhsT=wt[:, :64], rhs=wt[:], start=True, stop=True)

            # Per-k weight tiles: fine-grained deps let the k-th matmul start
            # as soon as chunk k lands.  Issued from the scalar engine so the
            # sync engine queue stays free for activations.
            w_sb = []
            for k in range(KT):
                t = wpool.tile([P, D], mm_dt, tag="w_sb")
                nc.scalar.dma_start(t[:], w[k, :, :])
                w_sb.append(t)
            # Activations: one contiguous DMA per n-tile.
            x_sb = []
            for j, (n0, nsz) in enumerate(n_tiles):
                t = xpool.tile([P, KT, nsz], mm_dt, tag="x_sb")
                nc.sync.dma_start(
                    t[:].rearrange("p k n -> p (k n)"),
                    xt[j, :, : KT * nsz],
                )
                x_sb.append(t)

            for mi in range(MT):
                ot = opool.tile([P, C], out_dt, tag="ot")
                for j, (n0, nsz) in enumerate(n_tiles):
                    ps = psum_pool.tile([P, NSPLIT], mybir.dt.float32, tag="ps")
                    for k in range(KT):
                        nc.tensor.matmul(
                            ps[:, :nsz],
                            lhsT=w_sb[k][:, mi * P : (mi + 1) * P],
                            rhs=x_sb[j][:, k, :nsz],
                            start=(k == 0),
                            stop=(k == KT - 1),
                        )
                    nc.vector.tensor_copy(ot[:, n0 : n0 + nsz], ps[:, :nsz])
                nc.sync.dma_start(yt[mi, :, :], ot[:])
    nc.compile()
    return nc


def kernel(hidden: np.ndarray, gate_logits: np.ndarray, W: np.ndarray) -> np.ndarray:
    from concourse.bass_utils import run_bass_kernel_spmd

    B, S, D = hidden.shape
    T, E = gate_logits.shape
    assert E == N_CORES
    x = np.ascontiguousarray(hidden.reshape(T, D).astype(np.float32))

    # --- routing on host (fp32, matches reference softmax/top-2) ---
    g = gate_logits.astype(np.float32)
    m = g.max(axis=-1, keepdims=True)
    p = np.exp(g - m)
    p /= p.sum(axis=-1, keepdims=True)
    top2 = np.argpartition(-p, 1, axis=-1)[:, :2]

    routed = [np.nonzero((top2 == e).any(axis=1))[0] for e in range(E)]
    counts = np.array([len(r) for r in routed])
    C = max(256, int(-(-counts.max() // P)) * P)  # capacity, multiple of 128

    mm_np = _np_dt(MM_DT)
    KT = D // P
    MT = D // P
    n_tiles = _n_tiles(C)
    NT = len(n_tiles)

    in_maps = []
    for e in range(E):
        idx = routed[e]
        scale = p[idx, e].astype(np.float32)
        xe = x[idx] * scale[:, None]  # [cnt, D]
        xt_full = np.zeros((D, C), dtype=mm_np)
        xt_full[:, : len(idx)] = xe.T.astype(mm_np)
        # [D, C] -> [KT, P, C] -> per-n-tile [NT, P, KT, nsz] contiguous
        xk = xt_full.reshape(KT, P, C)
        xt_dram = np.zeros((NT, P, KT * NSPLIT), dtype=mm_np)
        for j, (n0, nsz) in enumerate(n_tiles):
            blk = xk[:, :, n0 : n0 + nsz].transpose(1, 0, 2)  # [P, KT, nsz]
            xt_dram[j, :, : KT * nsz] = blk.reshape(P, KT * nsz)
        w_full = W[e].astype(mm_np)  # [D, D]
        in_maps.append(
            {
                "xt": np.ascontiguousarray(xt_dram),
                "w": np.ascontiguousarray(w_full.reshape(KT, P, D)),
            }
        )

    key = (D, C, MM_DT, OUT_DT, WARMUP_MM)
    if key not in _prog_cache:
        _prog_cache[key] = _build_program(D, C, MM_DT, OUT_DT)
    nc = _prog_cache[key]

    res = run_bass_kernel_spmd(nc, in_maps, core_ids=list(range(N_CORES)))

    # --- combine on host ---
    out = np.zeros((T, D), dtype=np.float32)
    for e in range(E):
        idx = routed[e]
        ye_t = res.results[e]["yt"].reshape(D, C)  # Y^T
        out[idx] += ye_t[:, : len(idx)].T.astype(np.float32)
    return out.reshape(B, S, D)
